# revision 20
# baseline (speedup 1.0000x reference)
"""Multi-head attention (B=4, S=2048, D=1024, H=16) on 8 trn2 NeuronCores.

Sharding: core c = 2*b + g handles batch b, head-group g (8 heads, 512 dims).
Q/K/V projections are column-sharded (Megatron), Wo row-sharded; the Wo
partial sums for the two head-groups of each batch are reduced host-side.

v2 design notes (vs the fp32r baseline):
  - All matmul operands are bf16 (host pre-casts inputs/weights).  Same PE
    cycles/column as fp32r but half the DMA + SBUF, no GPSIMD cast stage,
    no fp32r free>=256 rate cliff, and DVE 2x/4x modes on bf16 tiles.
  - PE p-state: the tensor engine only reaches 2.4 GHz after ~3us of
    gap-free execution, so the schedule is built to never starve the PE:
    each attention unit = one k-chunk with both head-halves packed in one
    [128, 1024] PSUM tile (2 banks), one exp per unit, PV lagged one unit
    behind scores, and projection matmul groups for the NEXT q-block (plus
    the PREVIOUS block's out-projection) interleaved as filler while ACT
    digests exp.
  - PSUM budget (8 banks): scores ring 2x[128,1024] (4) + PV accum
    2x[65,512] (2) + projection ring 2x[128,512] (2).
  - Causal: K/V projections interleaved per 512-row block, upper-triangle
    score chunks skipped, diagonal chunks zeroed multiplicatively post-exp
    (bf16 mask, DVE 4x).  Softmax denominators ride a ones-column in V.
"""

import numpy as np

import concourse.mybir as mybir
import concourse.tile as tile
from concourse import bacc
from concourse.bass_utils import run_bass_kernel_spmd

F32 = mybir.dt.float32
BF16 = mybir.dt.bfloat16
NPBF16 = mybir.dt.np(BF16)
B, S, D, H, DK = 4, 2048, 1024, 16, 64
DL = 512  # dims per head-group (8 heads * 64)
NH = 8  # heads per core
NHP = 4  # head pairs per core
NDI = D // 128  # 8   d_model chunks
NSC = S // 128  # 16  seq chunks (k side)
NSB = S // 512  # 4   seq blocks (q side)
NDC = DL // 128  # 4  local-dim chunks
NEG = -1.0e9


# ---------------------------------------------------------------------------
# device-side emission
# ---------------------------------------------------------------------------


def _emit_loads(env, qb):
    """Issue stream DMAs for block qb (kt/vt/qt), ring-2 buffered."""
    nc = env["nc"]
    ssl = slice(qb * 512, (qb + 1) * 512)
    for nm, dram in (("kt", env["kT_d"]), ("vt", env["vT_d"]), ("qt", env["qT_d"])):
        t = env["stream"].tile([128, NDI, 512], BF16, tag=f"{nm}s", name=f"{nm}{qb}")
        nc.sync.dma_start(
            out=t[:], in_=dram.ap()[:, ssl].rearrange("(c p) q -> p c q", p=128)
        )
        env[nm][qb] = t


def _kproj_group(env, sb, dc):
    nc = env["nc"]
    ssl = slice(sb * 512, (sb + 1) * 512)
    kt = env["kt"][sb]
    ps = env["ppp"].tile([128, 512], F32, tag="pp", name="ppk")
    for di in range(NDI):
        nc.tensor.matmul(
            ps[:],
            env["wk_r"][:, di, dc * 128 : (dc + 1) * 128],
            kt[:, di, :],
            start=(di == 0),
            stop=(di == NDI - 1),
        )
    nc.vector.tensor_scalar_add(
        env["Ksb"][:, dc, ssl], ps[:], env["bk_sb"][:, dc : dc + 1]
    )


def _qproj_group(env, qb, dc):
    nc = env["nc"]
    qt = env["qt"][qb]
    Qblk = env["Qblk"][qb]
    ps = env["ppp"].tile([128, 512], F32, tag="pp", name="ppq")
    for di in range(NDI):
        nc.tensor.matmul(
            ps[:],
            env["wq_r"][:, di, dc * 128 : (dc + 1) * 128],
            qt[:, di, :],
            start=(di == 0),
            stop=(di == NDI - 1),
        )
    nc.vector.tensor_scalar_add(Qblk[:, dc, :], ps[:], env["bq_sb"][:, dc : dc + 1])


def _vproj_group(env, sc):
    nc = env["nc"]
    vt = env["vt"][sc // 4]
    col = (sc % 4) * 128
    ps = env["ppp"].tile([128, DL], F32, tag="pp", name="ppv")
    for di in range(NDI):
        nc.tensor.matmul(
            ps[:],
            vt[:, di, col : col + 128],
            env["wv_r"][:, di, :],
            start=(di == 0),
            stop=(di == NDI - 1),
        )
    vt3 = env["vts"][sc][:].rearrange("p (h c) -> p h c", h=NH)
    nc.vector.tensor_add(
        vt3[:, :, 0:64],
        ps[:].rearrange("p (h c) -> p h c", h=NH),
        env["bv_sb"][:].rearrange("p (h c) -> p h c", h=NH),
    )
    nc.vector.tensor_copy(vt3[:, :, 64:65], env["ones_sb"][:].unsqueeze(2))


def _oproj_group(env, qb, ec):
    nc = env["nc"]
    qsl = slice(qb * 512, (qb + 1) * 512)
    Xblk = env["Xblk"][qb]
    ps = env["ppp"].tile([128, 512], F32, tag="pp", name="ppc")
    for dl in range(NDC):
        nc.tensor.matmul(
            ps[:],
            env["wo_r"][:, dl, ec * 128 : (ec + 1) * 128],
            Xblk[:, dl, :],
            start=(dl == 0),
            stop=(dl == NDC - 1),
        )
    ot = env["osp"].tile([128, 512], BF16, tag="ot", name="ot")
    nc.vector.tensor_copy(ot[:], ps[:])
    nc.scalar.dma_start(out=env["outT_d"].ap()[ec * 128 : (ec + 1) * 128, qsl], in_=ot[:])


def _emit_s_exp(env, qb, hp, kc, q0, skip_mask=False, skip_exp=False):
    """Scores for one (head-pair, k-chunk): both halves in one PSUM tile,
    one exp, optional multiplicative mask.  For causal diagonal chunks
    only the live q-range [q0:512) is computed; [q0:q0+128) gets the
    triangle mask.  Returns the bf16 es tile."""
    nc = env["nc"]
    variant = env["variant"]
    ps = env["pss"].tile([128, 1024], F32, tag="s", name="pss")
    for hb, (p0, p1) in enumerate(((0, 64), (64, 128))):
        nc.tensor.matmul(
            ps[:, hb * 512 + q0 : (hb + 1) * 512],
            env["Ksb"][p0:p1, hp, kc * 128 : (kc + 1) * 128],
            env["Qblk"][qb][p0:p1, hp, q0:],
            start=True,
            stop=True,
        )
    if skip_exp:
        return None
    et = env["ep"].tile([128, 1024], BF16, tag="e", name="et")
    ps3 = ps[:].rearrange("p (h q) -> p h q", h=2)
    et3 = et[:].rearrange("p (h q) -> p h q", h=2)
    nc.scalar.activation(
        et3[:, :, q0:],
        ps3[:, :, q0:],
        mybir.ActivationFunctionType.Exp,
        scale=1.0 / np.sqrt(DK),
    )
    if not skip_mask:
        if variant == "causal" and kc >= 4 * qb:
            nc.vector.tensor_mul(
                et3[:, :, q0 : q0 + 128],
                et3[:, :, q0 : q0 + 128],
                env["mt_sb"][:],
            )
        elif variant == "general":
            for hb in range(2):
                nc.vector.tensor_mul(
                    et[:, hb * 512 : (hb + 1) * 512],
                    et[:, hb * 512 : (hb + 1) * 512],
                    env["mq_sb"][qb % 2][:, kc, :],
                )
    return et


def _emit_pv(env, hp, kc, q0, first, last, es, ps_o):
    nc = env["nc"]
    for hb in range(2):
        h = 2 * hp + hb
        nc.tensor.matmul(
            ps_o[hb][:, q0:],
            env["vts"][kc][:, h * 65 : (h + 1) * 65],
            es[:, hb * 512 + q0 : (hb + 1) * 512],
            start=(kc == first),
            stop=(kc == last),
            skip_group_check=True,
        )


def _emit_normalize(env, qb, hp, ps_o):
    """Free the PV accumulators fast (recip + raw copy are the only PSUM
    reads); the broadcast + scale run later off the PE-critical path."""
    nc = env["nc"]
    Xblk = env["Xblk"][qb]
    for hb, (p0, p1) in enumerate(((0, 64), (64, 128))):
        r = env["rp"].tile([1, 512], F32, tag="r", name=f"r{hb}")
        xr = env["rp"].tile([64, 512], BF16, tag="xr", name=f"xr{hb}")
        rb = env["rp"].tile([64, 512], F32, tag="rb", name=f"rb{hb}")
        nc.vector.reciprocal(r[:], ps_o[hb][64:65, :])
        nc.vector.tensor_copy(xr[:], ps_o[hb][0:64, :])
        nc.gpsimd.partition_broadcast(rb[:], r[0:1, :])
        nc.vector.tensor_mul(Xblk[p0:p1, hp, :], xr[:], rb[:])


def _emit_attention(env, qb, filler):
    """Pipelined attention for block qb, draining `filler` (list of
    zero-arg closures emitting one PE matmul group each) at diagonal
    units and head-pair transitions."""
    nc = env["nc"]
    skip = env["skip"]
    kept = env["kept_kcs"](qb)
    first, last = kept[0], kept[-1]
    skip_pv = "pv" in skip or "exp" in skip
    skip_mask = "nomask" in skip or "exp" in skip
    skip_exp = "exp" in skip

    units = [(hp, kc) for hp in range(NHP) for kc in kept]
    fake_es = env.get("fake_es") if "fakepv" in skip else None
    fq = list(filler)
    fi = 0

    def pop_filler():
        nonlocal fi
        if fi < len(fq):
            fq[fi]()
            fi += 1

    # filler slots: before PV of diagonal units and at hp transitions
    def is_diag(kc):
        return env["variant"] == "causal" and kc >= 4 * qb

    def q0_of(kc):
        return 128 * (kc - 4 * qb) if is_diag(kc) else 0

    pend = None
    ps_os = {}
    for hp, kc in units:
        es = _emit_s_exp(
            env, qb, hp, kc, q0_of(kc), skip_mask=skip_mask, skip_exp=skip_exp
        )
        if fake_es is not None:
            es = fake_es
        if pend is not None:
            phw, pkc, pes = pend
            if is_diag(pkc) or pkc == last:
                pop_filler()
            if pkc == last:
                pop_filler()
            if not skip_pv:
                if pkc == first:
                    ps_os[phw] = {
                        0: env["pso"].tile([65, 512], F32, tag="oA", name="psoA"),
                        1: env["pso"].tile([65, 512], F32, tag="oB", name="psoB"),
                    }
                _emit_pv(env, phw, pkc, q0_of(pkc), first, last, pes, ps_os[phw])
                if pkc == last:
                    _emit_normalize(env, qb, phw, ps_os.pop(phw))
        pend = (hp, kc, es)
    if pend is not None and not skip_pv:
        phw, pkc, pes = pend
        pop_filler()
        if pkc == first:
            ps_os[phw] = {
                0: env["pso"].tile([65, 512], F32, tag="oA", name="psoA"),
                1: env["pso"].tile([65, 512], F32, tag="oB", name="psoB"),
            }
        _emit_pv(env, phw, pkc, q0_of(pkc), first, last, pes, ps_os[phw])
        _emit_normalize(env, qb, phw, ps_os.pop(phw))
    # leftover filler
    while fi < len(fq):
        fq[fi]()
        fi += 1


def build_program(variant, reps=1, skip=()):
    """variant: 'causal' | 'ones' | 'general'; skip: timing-ablation flags."""
    assert variant in ("causal", "ones", "general")
    nc = bacc.Bacc("TRN2", target_bir_lowering=False, debug=False)

    qT_d = nc.dram_tensor("qT", [D, S], BF16, kind="ExternalInput")
    kT_d = nc.dram_tensor("kT", [D, S], BF16, kind="ExternalInput")
    vT_d = nc.dram_tensor("vT", [D, S], BF16, kind="ExternalInput")
    wq_d = nc.dram_tensor("wq", [D, DL], BF16, kind="ExternalInput")
    wk_d = nc.dram_tensor("wk", [D, DL], BF16, kind="ExternalInput")
    wv_d = nc.dram_tensor("wv", [D, DL], BF16, kind="ExternalInput")
    wo_d = nc.dram_tensor("wo", [DL, D], BF16, kind="ExternalInput")
    bq_d = nc.dram_tensor("bq", [128, NDC], F32, kind="ExternalInput")
    bk_d = nc.dram_tensor("bk", [128, NDC], F32, kind="ExternalInput")
    bv_d = nc.dram_tensor("bv", [128, DL], F32, kind="ExternalInput")
    mt_d = mT_d = None
    if variant == "causal":
        # multiplicative 1/0 triangle tile [k, 2 halves, q_local]
        mt_d = nc.dram_tensor("maskt", [128, 2, 128], BF16, kind="ExternalInput")
    elif variant == "general":
        # multiplicative 1/0, transposed [k, q]
        mT_d = nc.dram_tensor("maskT", [S, S], BF16, kind="ExternalInput")
    outT_d = nc.dram_tensor("outT", [D, S], BF16, kind="ExternalOutput")

    def kept_kcs(qb):
        return list(range(4 * qb + 4)) if variant == "causal" else list(range(NSC))

    with tile.TileContext(nc) as tc:
        for _rep in range(reps):
            with (
                tc.tile_pool(name="persist", bufs=1) as pers,
                tc.tile_pool(name="stream", bufs=2) as stream,
                tc.tile_pool(name="qblk", bufs=2) as qbp,
                tc.tile_pool(name="xblk", bufs=3) as xbp,
                tc.tile_pool(name="epool", bufs=8) as ep,
                tc.tile_pool(name="rpool", bufs=2) as rp,
                tc.tile_pool(name="ostage", bufs=3) as osp,
                tc.tile_pool(name="mq", bufs=2) as mqp,
                tc.tile_pool(name="pss", bufs=2, space="PSUM") as pss,
                tc.tile_pool(name="pso", bufs=1, space="PSUM") as pso,
                tc.tile_pool(name="ppool", bufs=2, space="PSUM") as ppp,
            ):
                env = dict(
                    nc=nc, variant=variant, skip=skip, kept_kcs=kept_kcs,
                    qT_d=qT_d, kT_d=kT_d, vT_d=vT_d, mT_d=mT_d, outT_d=outT_d,
                    stream=stream, ep=ep, rp=rp, osp=osp, ppp=ppp,
                    pss=pss, pso=pso,
                    kt={}, vt={}, qt={}, Qblk={}, Xblk={}, mq_sb={},
                )
                # ---- constants (DMA queue order = first-needed first) ----
                bq_sb = pers.tile([128, NDC], F32, tag="bq", name="bq_sb")
                bk_sb = pers.tile([128, NDC], F32, tag="bk", name="bk_sb")
                bv_sb = pers.tile([128, DL], F32, tag="bv", name="bv_sb")
                ones_sb = pers.tile([128, NH], BF16, tag="ones", name="ones_sb")
                nc.any.memset(ones_sb[:], 1.0)
                env.update(bq_sb=bq_sb, bk_sb=bk_sb, bv_sb=bv_sb, ones_sb=ones_sb)

                def _load_w(wname, wd):
                    w_sb = pers.tile(
                        [128, NDI, DL], BF16, tag=wname, name=f"{wname}_sb"
                    )
                    nc.sync.dma_start(
                        out=w_sb[:],
                        in_=wd.ap().rearrange("(c p) l -> p c l", p=128),
                    )
                    env[f"{wname}_r"] = w_sb[:]

                def _load_stream1(nm, dram, qb):
                    t = env["stream"].tile(
                        [128, NDI, 512], BF16, tag=f"{nm}s", name=f"{nm}{qb}"
                    )
                    ssl = slice(qb * 512, (qb + 1) * 512)
                    nc.sync.dma_start(
                        out=t[:],
                        in_=dram.ap()[:, ssl].rearrange("(c p) q -> p c q", p=128),
                    )
                    env[nm][qb] = t

                _load_w("wk", wk_d)
                _load_stream1("kt", kT_d, 0)
                nc.sync.dma_start(out=bk_sb[:], in_=bk_d.ap())
                _load_w("wv", wv_d)
                _load_stream1("vt", vT_d, 0)
                nc.sync.dma_start(out=bv_sb[:], in_=bv_d.ap())
                if variant == "causal":
                    mt_sb = pers.tile([128, 2, 128], BF16, tag="mt", name="mt_sb")
                    nc.sync.dma_start(out=mt_sb[:], in_=mt_d.ap())
                    env["mt_sb"] = mt_sb
                _load_w("wq", wq_d)
                _load_stream1("qt", qT_d, 0)
                nc.sync.dma_start(out=bq_sb[:], in_=bq_d.ap())
                wo_sb = pers.tile([128, NDC, D], BF16, tag="wo", name="wo_sb")
                nc.sync.dma_start(
                    out=wo_sb[:], in_=wo_d.ap().rearrange("(c p) e -> p c e", p=128)
                )
                env["wo_r"] = wo_sb[:]

                if "fakepv" in skip:
                    fes = pers.tile([128, 1024], BF16, tag="fes", name="fes")
                    nc.any.memset(fes[:], 0.5)
                    env["fake_es"] = fes
                Ksb = pers.tile([128, NDC, S], BF16, tag="Ksb", name="Ksb")
                vts = [
                    pers.tile([128, NH * 65], BF16, tag=f"vt{sc}", name=f"vt{sc}")
                    for sc in range(NSC)
                ]
                env.update(Ksb=Ksb, vts=vts)

                for qb in range(NSB):
                    env["Qblk"][qb] = qbp.tile(
                        [128, NDC, 512], BF16, tag="Qblk", name=f"Qblk{qb}"
                    )
                    env["Xblk"][qb] = xbp.tile(
                        [128, NDC, 512], BF16, tag="Xblk", name=f"Xblk{qb}"
                    )

                def load_gen_mask(qb):
                    m = mqp.tile([128, NSC, 512], BF16, tag="mq", name=f"mq{qb}")
                    nc.sync.dma_start(
                        out=m[:],
                        in_=mT_d.ap()[:, qb * 512 : (qb + 1) * 512].rearrange(
                            "(c p) q -> p c q", p=128
                        ),
                    )
                    env["mq_sb"][qb % 2] = m

                causal = variant == "causal"
                # ---- prologue: block-1 loads, K/V proj, Q proj ----
                if not causal:
                    for sb in range(1, NSB):
                        _emit_loads(env, sb)
                else:
                    _emit_loads(env, 1)
                if variant == "general":
                    load_gen_mask(0)
                kblocks = [0] if causal else list(range(NSB))
                if "qk" not in skip:
                    for sb in kblocks:
                        for dc in range(NDC):
                            _kproj_group(env, sb, dc)
                if "v" not in skip:
                    for sb in kblocks:
                        for sc in range(4 * sb, 4 * sb + 4):
                            _vproj_group(env, sc)
                if "qk" not in skip:
                    for dc in range(NDC):
                        _qproj_group(env, 0, dc)

                # ---- main blocks ----
                for qb in range(NSB):
                    if causal and qb + 2 < NSB:
                        _emit_loads(env, qb + 2)
                    if variant == "general" and qb + 1 < NSB:
                        load_gen_mask(qb + 1)
                    filler = []
                    nqb = qb + 1
                    if nqb < NSB:
                        if causal and "qk" not in skip:
                            for dc in range(NDC):
                                filler.append(
                                    lambda s=nqb, d=dc: _kproj_group(env, s, d)
                                )
                        if causal and "v" not in skip:
                            for sc in range(4 * nqb, 4 * nqb + 4):
                                filler.append(lambda s=sc: _vproj_group(env, s))
                        if "qk" not in skip:
                            for dc in range(NDC):
                                filler.append(
                                    lambda q=nqb, d=dc: _qproj_group(env, q, d)
                                )
                    # out-proj filler: ride under block 3's ACT-bound shadow
                    omap = {2: [0], 3: [1, 2]} if causal else {
                        qb: [qb - 1] for qb in range(1, NSB)
                    }
                    if not ({"out", "pv", "attn", "exp"} & set(skip)):
                        for oq in omap.get(qb, []):
                            for ec in range(NDI):
                                filler.append(
                                    lambda q=oq, e=ec: _oproj_group(env, q, e)
                                )
                    if "attn" not in skip:
                        _emit_attention(env, qb, filler)
                    else:
                        for f in filler:
                            f()
                if not ({"out", "pv", "attn", "exp"} & set(skip)):
                    for ec in range(NDI):
                        _oproj_group(env, NSB - 1, ec)
    nc.compile()
    return nc


# ---------------------------------------------------------------------------
# host side
# ---------------------------------------------------------------------------

_NC_CACHE = {}


def _get_program(variant, reps=1):
    key = (variant, reps)
    if key not in _NC_CACHE:
        _NC_CACHE[key] = build_program(variant, reps)
    return _NC_CACHE[key]


def detect_variant(mask):
    m = np.asarray(mask)
    if (m != 0).all():
        return "ones"
    tril = np.tril(np.ones((S, S), np.int8))
    for b in range(m.shape[0]):
        mb = (m[b] != 0).astype(np.int8)
        if not np.array_equal(mb, tril):
            return "general"
    return "causal"


def make_causal_mask_tiles():
    k = np.arange(128)[:, None, None]
    q = np.arange(128)[None, None, :]
    # multiplicative: 1 keep, 0 drop (applied to exp'd scores); both halves
    return np.broadcast_to((q >= k), (128, 2, 128)).astype(NPBF16)


def build_in_maps(query, key, value, mask, Wq, bq, Wk, bk, Wv, bv, Wo, bo, variant):
    query = np.asarray(query, np.float32)
    key = np.asarray(key, np.float32)
    value = np.asarray(value, np.float32)
    Wq, Wk, Wv, Wo = (np.asarray(w, np.float32) for w in (Wq, Wk, Wv, Wo))
    bq, bk, bv = (np.asarray(x, np.float32) for x in (bq, bk, bv))

    if variant == "causal":
        mtiles = make_causal_mask_tiles()

    in_maps = []
    for c in range(8):
        b, g = c // 2, c % 2
        gs = slice(g * DL, (g + 1) * DL)
        m = {
            "qT": np.ascontiguousarray(query[b].T.astype(NPBF16)),
            "kT": np.ascontiguousarray(key[b].T.astype(NPBF16)),
            "vT": np.ascontiguousarray(value[b].T.astype(NPBF16)),
            "wq": np.ascontiguousarray(Wq[gs].T.astype(NPBF16)),
            "wk": np.ascontiguousarray(Wk[gs].T.astype(NPBF16)),
            "wv": np.ascontiguousarray(Wv[gs].T.astype(NPBF16)),
            "wo": np.ascontiguousarray(Wo[:, gs].T.astype(NPBF16)),
            "bq": np.ascontiguousarray(bq[gs].reshape(NDC, 128).T),
            "bk": np.ascontiguousarray(bk[gs].reshape(NDC, 128).T),
            "bv": np.ascontiguousarray(np.broadcast_to(bv[gs], (128, DL))),
        }
        if variant == "causal":
            m["maskt"] = mtiles
        elif variant == "general":
            m["maskT"] = np.ascontiguousarray(
                (np.asarray(mask[b]) != 0).astype(NPBF16).T
            )
        in_maps.append(m)
    return in_maps


def assemble_output(results, bo):
    bo = np.asarray(bo, np.float32)
    out = np.empty((B, S, D), np.float32)
    for b in range(B):
        acc = results[2 * b]["outT"].astype(np.float32) + results[
            2 * b + 1
        ]["outT"].astype(np.float32)
        out[b] = acc.T + bo
    return out


def kernel(query, key, value, mask, Wq, bq, Wk, bk, Wv, bv, Wo, bo):
    variant = detect_variant(np.asarray(mask))
    in_maps = build_in_maps(
        query, key, value, mask, Wq, bq, Wk, bk, Wv, bv, Wo, bo, variant
    )
    nc = _get_program(variant)
    res = run_bass_kernel_spmd(nc, in_maps, core_ids=list(range(8)))
    return assemble_output(res.results, bo)
